# revision 48
# baseline (speedup 1.0000x reference)
"""Cross-attention kernel for TRN2, 8 NeuronCores, data-parallel over points.

Math (derived from the reference):
  [qk | qp][n] = q[n] @ [MA | MQ]     MA = (Wq.T Wk)*s, MQ = Wq.T
  scores[n,w]  = qk[n] . k[w,n]
  attn[n]      = softmax_w(scores[n])
  vmixT        = sum_w v_w.T-weighted: vmixT[:,n] = sum_w attn[n,w] * v[w,n,:]
  y[n]         = gelu(vmixT.T @ MB + bo) + qp[n],  MB = Wv.T Wo.T
  out[c][8*i+j] = y[c*4096+i]   (row replication done on host)

Engine split per 128-point tile:
  PE:     q projections (host pre-transposes q, so no PE transposes),
          vmixT via diag(attn_w)-moving matmuls (v is the fp8 stationary),
          y = vmixT.T @ MB + bias matmul.
  DVE:    scores broadcast-mult (bf16 2x), fold cascade + final
          tensor_reduce, reciprocal (batched per group), diag build
          (8x tensor_scalar with e*rs fused), residual add.
  ACT:    one merged [qk|qp] PSUM->SBUF cast, vmixT cast, Exp (fused
          denominator), Gelu per group.
GpSimd is intentionally UNUSED: every GpSimd op arbitrates for the SBUF
shared port pair that DVE 2-port ops hold, so GpSimd work serializes
against the DVE stream instead of overlapping.
DMA traffic per core: k bf16 16MB, v fp8 8MB, q bf16 2MB, out bf16 2MB.
"""

import ml_dtypes
import numpy as np

import concourse.bass as bass
import concourse.mybir as mybir
import concourse.tile as tile
from concourse import bacc
from concourse.bass_utils import run_bass_kernel_spmd

N_CORES = 8
N_TOTAL = 32768
NC_PTS = N_TOTAL // N_CORES  # 4096 points per core
D = 256
V = 8
P = 128
G = 4  # tiles per group (gelu batching + q/out DMA batching)
N_TILES = NC_PTS // P  # 32
F32 = mybir.dt.float32
BF16 = mybir.dt.bfloat16
FP8 = mybir.dt.float8e4
NP_BF16 = ml_dtypes.bfloat16
NP_FP8 = ml_dtypes.float8_e4m3
AX = mybir.AxisListType
OP = mybir.AluOpType
AF = mybir.ActivationFunctionType


def _bcast(ap, axis_count, after_dims):
    """Insert a [0, axis_count] broadcast dim before the last `after_dims`
    dims of `ap`'s access pattern."""
    dims = list(ap.ap)
    pos = len(dims) - after_dims
    dims = dims[:pos] + [[0, axis_count]] + dims[pos:]
    return bass.AP(tensor=ap.tensor, offset=ap.offset, ap=dims)


def _pairview(ap, lo, n):
    """[P, 2, V, D] k-pair tile -> 3D [P, 2*V, n] view at column offset lo.
    Legal because stride(s) == V*stride(v); keeping the AP 3-dim preserves
    the DVE 2x perf mode (4-dim APs were measured to drop it)."""
    d = list(ap.ap)
    assert len(d) == 4 and d[1][0] == d[2][1] * d[2][0]
    return bass.AP(
        tensor=ap.tensor,
        offset=ap.offset + lo,
        ap=[d[0], [d[2][0], d[1][1] * d[2][1]], [d[3][0], n]],
    )


def build_bass(n_tiles: int = N_TILES, gelu: bool = True):
    nc = bacc.Bacc(
        "TRN2", target_bir_lowering=False, debug=False, num_devices=N_CORES
    )
    assert n_tiles % G == 0
    n_groups = n_tiles // G
    npts = n_tiles * P
    # q pre-transposed on host: [d-half(part), 2, npts]
    q_d = nc.dram_tensor("qT", [P, 2, npts], BF16, kind="ExternalInput")
    k_d = nc.dram_tensor("k", [npts, V, D], BF16, kind="ExternalInput")
    v_d = nc.dram_tensor("v", [npts, V, D], FP8, kind="ExternalInput")
    mamq_d = nc.dram_tensor("mamq", [2, P, 2 * D], BF16, kind="ExternalInput")
    mb_d = nc.dram_tensor("mb", [2, P, D], BF16, kind="ExternalInput")
    bo_d = nc.dram_tensor("bo_r", [1, D], BF16, kind="ExternalInput")
    ones_d = nc.dram_tensor("ones_r", [1, P], BF16, kind="ExternalInput")
    id8_d = nc.dram_tensor("id8", [P, V, P], BF16, kind="ExternalInput")
    # unique rows only, packed [P, n_tiles, D] bf16; host replicates 8x
    out_d = nc.dram_tensor("out", [P, n_tiles, D], BF16, kind="ExternalOutput")

    with tile.TileContext(nc) as tc:
        with (
            tc.tile_pool(name="singles", bufs=1) as singles,
            tc.tile_pool(name="qio", bufs=2) as qio,
            tc.tile_pool(name="io", bufs=8) as io,
            tc.tile_pool(name="work", bufs=4) as work,
            tc.tile_pool(name="tl", bufs=6) as tl,
            tc.tile_pool(name="gwork", bufs=2) as gwork,
            tc.tile_pool(name="ps", bufs=2, space="PSUM") as ps,
            tc.tile_pool(name="psy", bufs=2, space="PSUM") as psy,
        ):
            mamq_t = singles.tile([P, 2, 2 * D], BF16)
            mb_t = singles.tile([P, 2, D], BF16)
            bo_t = singles.tile([1, D], BF16)
            ones_t = singles.tile([1, P], BF16)
            id8_t = singles.tile([P, V, P], BF16)
            for h in range(2):
                nc.sync.dma_start(out=mamq_t[:, h], in_=mamq_d.ap()[h])

            def emit_qkqp_group(gq):
                """q DMA + [qk|qp] projections + cast for group gq. Emitted
                one group AHEAD so the score mults never wait on the ACT
                queue draining the previous group's epilogue."""
                gq0 = gq * G
                qT_g = qio.tile([P, 2, G * P], BF16, tag="q")
                nc.sync.dma_start(
                    out=qT_g, in_=q_d.ap()[:, :, gq0 * P : (gq0 + G) * P]
                )
                bs = []
                for t in range(G):
                    if t % 2 == 0:
                        qkqp_b2 = tl.tile([P, 2, 2 * D], BF16, tag="qkqp_b")
                        bs.append(qkqp_b2)
                    qkqp_ps = ps.tile([P, 2 * D], F32, tag="qkqp")
                    nc.tensor.matmul(
                        qkqp_ps,
                        qT_g[:, 0, t * P : (t + 1) * P],
                        mamq_t[:, 0],
                        start=True,
                        stop=False,
                    )
                    nc.tensor.matmul(
                        qkqp_ps,
                        qT_g[:, 1, t * P : (t + 1) * P],
                        mamq_t[:, 1],
                        start=False,
                        stop=True,
                    )
                    nc.scalar.copy(bs[t // 2][:, t % 2], qkqp_ps)
                return bs

            def emit_k_group(gq):
                """Two slice-DMAs per pair tile: the first score mult then
                waits on 512KB, not the whole 1MB pair (the SDMA engines
                round-robin all queues, so everything in flight finishes
                together — smaller completion units start compute sooner)."""
                ks = []
                for half in range(G // 2):
                    gq0 = gq * G
                    k2_t = io.tile([P, 2, V, D], BF16, tag="k")
                    for u in range(2):
                        ti = gq0 + 2 * half + u
                        nc.sync.dma_start(
                            out=k2_t[:, u],
                            in_=k_d.ap()[ti * P : (ti + 1) * P],
                        )
                    ks.append(k2_t)
                return ks

            # startup-critical first: the q-projection chain (mamq is
            # already in flight), then group 0's k tiles; the y-stage
            # constants can land much later
            cur_bs = emit_qkqp_group(0)
            k2_first = emit_k_group(0)
            nc.sync.dma_start(
                out=mb_t, in_=mb_d.ap().rearrange("h p d -> p h d")
            )
            nc.sync.dma_start(out=bo_t, in_=bo_d.ap())
            nc.sync.dma_start(out=ones_t, in_=ones_d.ap())
            nc.sync.dma_start(out=id8_t, in_=id8_d.ap())
            pending = None

            for gi in range(n_groups):
                g0 = gi * G
                y_ps = psy.tile([P, G, D], F32, tag="y")
                qkqp_bs = cur_bs
                e_ts = []
                sm_g = work.tile([P, G], F32, tag="smg")
                rs_g = work.tile([P, G], F32, tag="rsg")

                k2_ts = k2_first if gi == 0 else emit_k_group(gi)

                if gi + 1 < n_groups:
                    next_bs = emit_qkqp_group(gi + 1)
                else:
                    next_bs = None

                for s in range(G // 2):
                    k2 = k2_ts[s]
                    qb2 = qkqp_bs[s]

                    # bcast mults stay per-tile (a 4-dim paired AP drops the
                    # 2x mode); folds/reduce run per PAIR via 3D merged views
                    for u in range(2):
                        nc.vector.tensor_tensor(
                            k2[:, u],
                            _bcast(qb2[:, u, 0:D], V, 1),
                            k2[:, u],
                            op=OP.mult,
                        )
                    k2ap = k2[:, 0:2, 0:V, 0:D]
                    nc.vector.tensor_tensor(
                        _pairview(k2ap, 0, 128),
                        _pairview(k2ap, 0, 128),
                        _pairview(k2ap, 128, 128),
                        op=OP.add,
                    )
                    nc.vector.tensor_tensor(
                        _pairview(k2ap, 0, 64),
                        _pairview(k2ap, 0, 64),
                        _pairview(k2ap, 64, 64),
                        op=OP.add,
                    )
                    nc.vector.tensor_tensor(
                        _pairview(k2ap, 0, 32),
                        _pairview(k2ap, 0, 32),
                        _pairview(k2ap, 32, 32),
                        op=OP.add,
                    )
                    scores2 = work.tile([P, 2, V], F32, tag="scores")
                    nc.vector.tensor_reduce(
                        scores2, _pairview(k2ap, 0, 32), axis=AX.X, op=OP.add
                    )

                    # softmax (no max-shift: scores ~N(0,1), f32 exp safe)
                    e2 = work.tile([P, 2, V], F32, tag="e")
                    for u in range(2):
                        t = 2 * s + u
                        nc.scalar.activation(
                            e2[:, u],
                            scores2[:, u],
                            AF.Exp,
                            accum_out=sm_g[:, t : t + 1],
                        )
                    e_ts.append(e2)

                # one batched reciprocal for the group's denominators
                nc.vector.reciprocal(rs_g, sm_g)

                # previous group's residual add + store: its gelu finished
                # a group ago, so these never stall the DVE FIFO
                if pending is not None:
                    p_gl, p_bs, p_g0 = pending
                    y_out = gwork.tile([P, G, D], BF16, tag="yout")
                    for s2 in range(G // 2):
                        nc.vector.tensor_tensor(
                            y_out[:, 2 * s2 : 2 * s2 + 2],
                            p_gl[:, 2 * s2 : 2 * s2 + 2],
                            p_bs[s2][:, :, D : 2 * D],
                            op=OP.add,
                        )
                    nc.scalar.dma_start(
                        out=out_d.ap()[:, p_g0 : p_g0 + G], in_=y_out
                    )

                v2_ts = []
                for t in range(G):
                    ti = g0 + t
                    if t % 2 == 0:
                        sl2 = slice(ti * P, (ti + 2) * P)
                        v2_t = io.tile([P, 2, V, D], FP8, tag="v")
                        nc.sync.dma_start(
                            out=v2_t,
                            in_=v_d.ap()[sl2].rearrange(
                                "(s p) v d -> p s v d", p=P
                            ),
                        )
                        v2_ts.append(v2_t)
                    v_t = v2_ts[t // 2][:, t % 2]

                    # diag = (id8 * rs) * e in ONE DVE op (per-op overhead
                    # dominates many small TSs); [P, V, P] layout keeps the
                    # PE moving operand slices contiguous
                    diag_t = work.tile([P, V, P], BF16, tag="diag")
                    nc.vector.scalar_tensor_tensor(
                        out=diag_t,
                        in0=id8_t,
                        scalar=rs_g[:, t : t + 1],
                        in1=_bcast(e_ts[t // 2][:, t % 2], P, 0),
                        op0=OP.mult,
                        op1=OP.mult,
                    )

                    # vmixT[d,n] = sum_w v_w[n,d]*attn_w[n]: stat = v (fp8)
                    vmixT_ps = ps.tile([P, 2, P], F32, tag="vmixT")
                    for h in range(2):
                        for w in range(V):
                            nc.tensor.matmul(
                                vmixT_ps[:, h],
                                v_t[:, w, h * P : (h + 1) * P],
                                diag_t[:, w],
                                start=(w == 0),
                                stop=(w == V - 1),
                            )
                    vT_b = work.tile([P, 2, P], BF16, tag="vT")
                    nc.scalar.copy(vT_b, vmixT_ps)

                    # y = vmixT.T @ MB + bo (bias as a K=1 matmul)
                    nc.tensor.matmul(
                        y_ps[:, t], ones_t, bo_t, start=True, stop=False
                    )
                    nc.tensor.matmul(
                        y_ps[:, t], vT_b[:, 0], mb_t[:, 0], start=False, stop=False
                    )
                    nc.tensor.matmul(
                        y_ps[:, t], vT_b[:, 1], mb_t[:, 1], start=False, stop=True
                    )

                    # LAST group: drain the epilogue per pair so the final
                    # gelu/add/store pipeline with the remaining vmix work
                    # instead of serializing after it (trims the kernel tail)
                    if gi == n_groups - 1 and t % 2 == 1:
                        s2 = t // 2
                        gl_h = gwork.tile([P, 2, D], BF16, tag="glh")
                        nc.scalar.activation(
                            gl_h,
                            y_ps[:, 2 * s2 : 2 * s2 + 2],
                            AF.Gelu if gelu else AF.Identity,
                        )
                        yo_h = gwork.tile([P, 2, D], BF16, tag="yoh")
                        nc.vector.tensor_tensor(
                            yo_h,
                            gl_h,
                            qkqp_bs[s2][:, :, D : 2 * D],
                            op=OP.add,
                        )
                        nc.scalar.dma_start(
                            out=out_d.ap()[:, g0 + 2 * s2 : g0 + 2 * s2 + 2],
                            in_=yo_h,
                        )

                if gi == n_groups - 1:
                    continue
                # gelu now; the residual add + store runs early next group
                gl = gwork.tile([P, G, D], BF16, tag="gl")
                nc.scalar.activation(gl, y_ps, AF.Gelu if gelu else AF.Identity)
                pending = (gl, qkqp_bs, g0)
                cur_bs = next_bs

    nc.compile()
    return nc


_NC_CACHE = {}


def _get_nc(n_tiles: int = N_TILES):
    if n_tiles not in _NC_CACHE:
        _NC_CACHE[n_tiles] = build_bass(n_tiles)
    return _NC_CACHE[n_tiles]


def _host_prep(Wq, Wk, Wv, Wo, bo):
    Wq = np.asarray(Wq, dtype=np.float32)
    Wk = np.asarray(Wk, dtype=np.float32)
    Wv = np.asarray(Wv, dtype=np.float32)
    Wo = np.asarray(Wo, dtype=np.float32)
    bo = np.asarray(bo, dtype=np.float32)
    scale = np.float32(1.0) / np.sqrt(np.float32(D))
    ma = (Wq.T @ Wk) * scale
    mq = Wq.T
    mamq = (
        np.concatenate([ma, mq], axis=1).reshape(2, P, 2 * D).astype(NP_BF16)
    )
    mb = (Wv.T @ Wo.T).reshape(2, P, D).astype(NP_BF16)
    bo_r = bo.reshape(1, D).astype(NP_BF16)
    ones_r = np.ones((1, P), dtype=NP_BF16)
    id8 = np.ascontiguousarray(
        np.broadcast_to(np.eye(P, dtype=np.float32)[:, None, :], (P, V, P))
    ).astype(NP_BF16)
    return (
        np.ascontiguousarray(mamq),
        np.ascontiguousarray(mb),
        bo_r,
        ones_r,
        id8,
    )


def make_in_maps(q, k, v, Wq, Wk, Wv, Wo, bo):
    q = np.asarray(q, dtype=np.float32)
    k = np.asarray(k, dtype=np.float32)
    v = np.asarray(v, dtype=np.float32)
    mamq, mb, bo_r, ones_r, id8 = _host_prep(Wq, Wk, Wv, Wo, bo)
    in_maps = []
    for c in range(N_CORES):
        sl = slice(c * NC_PTS, (c + 1) * NC_PTS)
        # qT: [128(d-half part), 2, npts]
        q_c = np.ascontiguousarray(
            q[0, sl].T.reshape(2, P, NC_PTS).transpose(1, 0, 2)
        ).astype(NP_BF16)
        k_c = k[:, sl].transpose(1, 0, 2).astype(NP_BF16)
        v_c = v[:, sl].transpose(1, 0, 2).astype(NP_FP8)
        in_maps.append(
            {
                "qT": q_c,
                "k": np.ascontiguousarray(k_c),
                "v": np.ascontiguousarray(v_c),
                "mamq": mamq,
                "mb": mb,
                "bo_r": bo_r,
                "ones_r": ones_r,
                "id8": id8,
            }
        )
    return in_maps


def gather_out(results):
    """[P, N_TILES, D] bf16 per core -> [8, 32768, 256] f32, 8x replicated."""
    out = np.empty((N_CORES, N_TOTAL, D), dtype=np.float32)
    for c in range(N_CORES):
        y = (
            results[c]["out"]
            .astype(np.float32)
            .transpose(1, 0, 2)
            .reshape(NC_PTS, D)
        )
        out[c] = np.repeat(y, V, axis=0)
    return out


def kernel(q, k, v, Wq, Wk, Wv, Wo, bo):
    nc = _get_nc()
    in_maps = make_in_maps(q, k, v, Wq, Wk, Wv, Wo, bo)
    res = run_bass_kernel_spmd(nc, in_maps, core_ids=list(range(N_CORES)))
    return gather_out(res.results)
